# revision 2
# baseline (speedup 1.0000x reference)
"""Causal attention kernel for Trainium2, 8 NeuronCores.

Problem: x[4, 2048, 1024], Wq/Wk/Wv[1024, 1024] (stored as [d_in, d_out]):
    q = x @ Wq; k = x @ Wk; v = x @ Wv
    out = softmax(causal(q @ k^T) / sqrt(1024)) @ v

Sharding: 8 cores = 4 batches x 2 query-sets. Core (b, t) handles batch b and
the interleaved global query blocks {2j + t : j in 0..7} (128 rows each).

Structure: scores are computed TRANSPOSED (S^T[k, q] via lhsT=KT-block,
rhs=QT-block) so the exp activation writes P^T straight to SBUF — exactly the
stationary operand the AV matmul needs; no PE transposes of P. The softmax
denominator accumulates via a ones-column matmul on the same P^T tiles.

The PE instruction stream is INTERLEAVED: Q projection first, then per
512-key chunk the K projection (key-major), the V projection for those key
blocks, and then every attention q-block whose causal extent is now covered.
Attention runs ascending so the final PE work is block 7's AV (fed by
already-computed exps), shrinking the end-of-kernel drain; early blocks
overlap the remaining projections.

Precision: bf16 PE matmuls with fp32 PSUM accumulation; softmax without
max-subtraction (scores ~N(0,1) after the folded 1/sqrt(D) scale).
"""

import numpy as np
import ml_dtypes
from contextlib import ExitStack

import concourse.bacc as bacc
import concourse.tile as tile
from concourse.tile import add_dep_helper
from concourse import mybir
from concourse.bass_utils import run_bass_kernel_spmd

B = 4          # batch
S = 2048       # sequence length
D = 1024       # d_in = d_out
NCORES = 8
QB = 128       # query block rows
NQB = S // QB // 2   # 8 q-blocks per core
SQ = NQB * QB        # 1024 query rows per core
SC = 512             # psum/projection chunk width
NDC = D // 128       # 8 contraction chunks
SCALE = 1.0 / float(np.sqrt(D))
MASK_VAL = -1e10

BF = mybir.dt.bfloat16
F32 = mybir.dt.float32


def build_program():
    nc = bacc.Bacc("TRN2", target_bir_lowering=False, debug=False,
                   num_devices=NCORES)

    xt_d = nc.dram_tensor("xt", [128, S // SC, NDC, SC], BF,
                          kind="ExternalInput")
    xqt_d = nc.dram_tensor("xqt", [128, SQ // SC, NDC, SC], BF,
                           kind="ExternalInput")
    wq_d = nc.dram_tensor("wq", [128, NDC, NDC, 128], BF, kind="ExternalInput")
    wk_d = nc.dram_tensor("wk", [128, NDC, D // 2], BF, kind="ExternalInput")
    wv_d = nc.dram_tensor("wv", [128, NDC, D], BF, kind="ExternalInput")
    # transposed causal mask for the diagonal key-pair: [k%128, k//128, q]
    msk_d = nc.dram_tensor("msk", [128, 2, QB], F32, kind="ExternalInput")
    out_d = nc.dram_tensor("out", [NQB, QB, D], F32, kind="ExternalOutput")

    with tile.TileContext(nc) as tc, ExitStack() as ctx:
        consts = ctx.enter_context(tc.tile_pool(name="consts", bufs=1))
        persist = ctx.enter_context(tc.tile_pool(name="persist", bufs=1))
        ps_pool = ctx.enter_context(
            tc.tile_pool(name="ps_pool", bufs=1, space="PSUM"))
        pt_pool = ctx.enter_context(tc.tile_pool(name="pt_sb_pool", bufs=20))
        att_sb = ctx.enter_context(tc.tile_pool(name="att_sb", bufs=2))
        stat_sb = ctx.enter_context(tc.tile_pool(name="stat_sb", bufs=4))

        msk_sb = consts.tile([128, 2, QB], F32, name="msk_sb")
        nc.sync.dma_start(out=msk_sb, in_=msk_d.ap())
        ones_sb = consts.tile([128, 1], BF, name="ones_sb")
        nc.vector.memset(ones_sb, 1.0)
        warm_sb = consts.tile([128, SC], BF, name="warm_sb")
        nc.vector.memset(warm_sb, 0.125)

        # Persistent activations (partition = head dim for QT/KT, = keys for V)
        QT = persist.tile([128, NDC, SQ], BF, name="QT")   # Q^T, pre-scaled
        KT = persist.tile([128, NDC, S], BF, name="KT")    # K^T
        V = persist.tile([128, S // 128, D], BF, name="V")  # V rows

        dram = ctx.enter_context(tc.tile_pool(name="dram", bufs=1, space="DRAM"))
        proj_sb = ctx.enter_context(tc.tile_pool(name="proj_sb", bufs=1))
        xT = proj_sb.tile([128, S // SC, NDC, SC], BF, name="xT")
        xqT = proj_sb.tile([128, SQ // SC, NDC, SC], BF, name="xqT")
        wq_sb = proj_sb.tile([128, NDC, NDC, 128], BF, name="wq_sb")
        wk_sb = proj_sb.tile([128, NDC, D // 2], BF, name="wk_sb")
        wv_sb = proj_sb.tile([128, NDC, D], BF, name="wv_sb")

        # PE warmup: matmuls on a memset tile run while the input DMAs
        # stream, releasing the HAM clock gate before real work.
        warm_ps = ps_pool.tile([128, SC], F32, name="warm_ps", tag="pp", bufs=2)
        for _ in range(10):
            nc.tensor.matmul(
                warm_ps, lhsT=warm_sb[:, 0:128], rhs=warm_sb,
                start=True, stop=True,
            )

        # DMA priority order: the K projection runs first (its chunks feed
        # the pairwise exchange), so Wk + x^T lead; Q inputs stream next
        # while K computes; Wv last (V projection starts after Q).
        nc.sync.dma_start(out=wk_sb, in_=wk_d.ap())
        xlast = None
        for sh in range(S // SC):
            xlast = nc.sync.dma_start(out=xT[:, sh], in_=xt_d.ap()[:, sh])
        i = nc.sync.dma_start(out=wq_sb[:, 0], in_=wq_d.ap()[:, 0])
        add_dep_helper(i.ins, xlast.ins, reason="dma phase order")
        i = nc.sync.dma_start(out=xqT[:, 0], in_=xqt_d.ap()[:, 0])
        add_dep_helper(i.ins, xlast.ins, reason="dma phase order")
        for ec in range(1, NDC):
            i = nc.sync.dma_start(out=wq_sb[:, ec], in_=wq_d.ap()[:, ec])
            add_dep_helper(i.ins, xlast.ins, reason="dma phase order")
        qlast = nc.sync.dma_start(out=xqT[:, 1], in_=xqt_d.ap()[:, 1])
        add_dep_helper(qlast.ins, xlast.ins, reason="dma phase order")
        i = nc.sync.dma_start(out=wv_sb, in_=wv_d.ap())
        add_dep_helper(i.ins, qlast.ins, reason="dma phase order")

        # ---- K^T e-half projection + pairwise exchange.
        # Each core projects only the 512 head dims the host staged in its
        # wk (even cores: dims 0..511, odd: 512..1023); a per-KEY-CHUNK
        # AllGather assembles the full K^T in HBM and both cores read back
        # both slots into KT in GLOBAL dim order. Chunking by key range
        # means attention block j only depends on the first j//2+1
        # exchanges, so early blocks never wait for the CC tail.
        KTtmp = proj_sb.tile([128, S // SC, NDC // 2, SC], BF, name="KTtmp")
        for sh in range(S // SC):
            for ec in range(NDC // 2):
                pp = ps_pool.tile([128, SC], F32, name="pp", tag="pp", bufs=2)
                for dc in range(NDC):
                    nc.tensor.matmul(
                        pp,
                        lhsT=wk_sb[:, dc, ec * 128:(ec + 1) * 128],
                        rhs=xT[:, sh, dc, :],
                        start=(dc == 0),
                        stop=(dc == NDC - 1),
                    )
                nc.scalar.copy(KTtmp[:, sh, ec, :], pp)
            cc_in = dram.tile([128, NDC // 2, SC], BF, name=f"cc_in{sh}")
            cc_out = dram.tile([2, 128, NDC // 2, SC], BF, name=f"cc_out{sh}")
            nc.gpsimd.dma_start(out=cc_in[:], in_=KTtmp[:, sh])
            nc.gpsimd.collective_compute(
                "AllGather",
                mybir.AluOpType.bypass,
                replica_groups=[[0, 1], [2, 3], [4, 5], [6, 7]],
                ins=[cc_in.opt()],
                outs=[cc_out.opt()],
            )
            shsl = slice(sh * SC, (sh + 1) * SC)
            nc.gpsimd.dma_start(out=KT[:, 0:NDC // 2, shsl], in_=cc_out[0])
            nc.gpsimd.dma_start(out=KT[:, NDC // 2:NDC, shsl], in_=cc_out[1])

        # ---- Q^T[e, s] = sum_d Wq[d, e] * xq^T[d, s]   (scale folded in)
        for sh in range(SQ // SC):
            for ec in range(NDC):
                pp = ps_pool.tile([128, SC], F32, name="pp", tag="pp", bufs=2)
                for dc in range(NDC):
                    nc.tensor.matmul(
                        pp,
                        lhsT=wq_sb[:, ec, dc, :],
                        rhs=xqT[:, sh, dc, :],
                        start=(dc == 0),
                        stop=(dc == NDC - 1),
                    )
                nc.scalar.mul(QT[:, ec, sh * SC:(sh + 1) * SC], pp, SCALE)

        def v_proj(kb):
            for eh in range(D // SC):
                pp = ps_pool.tile([128, SC], F32, name="pp", tag="pp", bufs=2)
                for dc in range(NDC):
                    nc.tensor.matmul(
                        pp,
                        lhsT=xT[:, kb // 4, dc,
                                (kb % 4) * 128:(kb % 4 + 1) * 128],
                        rhs=wv_sb[:, dc, eh * SC:(eh + 1) * SC],
                        start=(dc == 0),
                        stop=(dc == NDC - 1),
                    )
                nc.scalar.copy(V[:, kb, eh * SC:(eh + 1) * SC], pp)

        def attention(j):
            nkb = (j + 1) * 2             # causal key extent in 128-blocks
            qsl = slice(j * 128, (j + 1) * 128)
            den_ps = ps_pool.tile([128, 1], F32, name="den_ps", tag="den", bufs=1)
            ps_av = [
                ps_pool.tile([128, SC], F32, name="ps_av", tag="ps_av", bufs=2)
                for _ in range(D // SC)
            ]
            for kp in range(j + 1):
                ps2 = ps_pool.tile([128, 2, 128], F32, name="ps2",
                                   tag="ps2", bufs=3)
                for i in range(2):
                    kb = 2 * kp + i
                    for ec in range(NDC):
                        nc.tensor.matmul(
                            ps2[:, i, :],
                            lhsT=KT[:, ec, kb * 128:(kb + 1) * 128],
                            rhs=QT[:, ec, qsl],
                            start=(ec == 0),
                            stop=(ec == NDC - 1),
                        )
                if kp == j:
                    # causal mask on the diagonal key-pair
                    nc.vector.tensor_add(out=ps2, in0=ps2, in1=msk_sb)
                pt = pt_pool.tile([128, 2, 128], BF, name="pt", tag="pt")
                nc.scalar.activation(
                    pt, ps2, mybir.ActivationFunctionType.Exp,
                    bias=0.0, scale=1.0,
                )
                for i in range(2):
                    kb = 2 * kp + i
                    nc.tensor.matmul(
                        den_ps,
                        lhsT=pt[:, i, :],
                        rhs=ones_sb,
                        start=(kb == 0),
                        stop=(kb == nkb - 1),
                    )
                    for eh in range(D // SC):
                        nc.tensor.matmul(
                            ps_av[eh],
                            lhsT=pt[:, i, :],
                            rhs=V[:, kb, eh * SC:(eh + 1) * SC],
                            start=(kb == 0),
                            stop=(kb == nkb - 1),
                        )

            rinv = stat_sb.tile([128, 1], F32, name="rinv", tag="rinv")
            nc.vector.reciprocal(rinv, den_ps)
            ob = att_sb.tile([128, D], F32, name="ob", tag="ob")
            for eh in range(D // SC):
                nc.scalar.mul(ob[:, eh * SC:(eh + 1) * SC], ps_av[eh], rinv)
                nc.sync.dma_start(
                    out=out_d.ap()[j][:, eh * SC:(eh + 1) * SC],
                    in_=ob[:, eh * SC:(eh + 1) * SC],
                )

        # Interleaved stream: per 512-key chunk project V, then run the
        # attention blocks whose extent was covered one stage earlier —
        # the one-stage delay gives each key-chunk's K^T exchange a full
        # projection stage of slack before its first consumer.
        for sh in range(S // SC):
            for kb in range(4 * sh, 4 * sh + 4):
                v_proj(kb)
            if sh > 0:
                for j in range(2 * sh - 2, 2 * sh):
                    attention(j)
        for j in range(2 * (S // SC) - 2, NQB):
            attention(j)

    nc.compile()
    return nc


_PROGRAM = None


def _get_program():
    global _PROGRAM
    if _PROGRAM is None:
        _PROGRAM = build_program()
    return _PROGRAM


def _pack_w(w):
    # [D, D] -> [128, NDC, D]: partition p, chunk dc holds row dc*128+p
    bf = ml_dtypes.bfloat16
    return np.ascontiguousarray(
        w.astype(bf).reshape(NDC, 128, D).transpose(1, 0, 2)
    )


def _pack_xt(xr):
    # [rows, D] -> x^T packed [128, NDC, rows]
    bf = ml_dtypes.bfloat16
    return np.ascontiguousarray(
        xr.astype(bf).T.reshape(NDC, 128, xr.shape[0]).transpose(1, 0, 2)
    )


def make_in_maps(x, Wq, Wk, Wv):
    # wq: [128, ec, dc, 128] so each ec-chunk is one small priority DMA
    wqb = np.ascontiguousarray(
        _pack_w(Wq).reshape(128, NDC, NDC, 128).transpose(0, 2, 1, 3)
    )
    wkb_full = _pack_w(Wk)
    wvb = _pack_w(Wv)
    r = np.arange(QB)[None, None, :]
    kk = (np.arange(2)[None, :, None] * 128 + np.arange(128)[:, None, None])
    in_maps = []
    for c in range(NCORES):
        b, t = c // 2, c % 2
        xb = x[b]
        xqb = xb.reshape(S // QB, QB, D)[t::2].reshape(SQ, D)
        xqtb = np.ascontiguousarray(
            _pack_xt(xqb).reshape(128, NDC, SQ // SC, SC).transpose(0, 2, 1, 3)
        )
        # transposed mask: mskT[k%128, k//128, q] over the diagonal key-pair
        mask = np.where(kk <= t * QB + r, 0.0, MASK_VAL).astype(np.float32)
        wkb = np.ascontiguousarray(wkb_full[:, :, t * (D // 2):(t + 1) * (D // 2)])
        xtb = np.ascontiguousarray(
            _pack_xt(xb).reshape(128, NDC, S // SC, SC).transpose(0, 2, 1, 3)
        )
        in_maps.append(
            {"xt": xtb, "xqt": xqtb,
             "wq": wqb, "wk": wkb, "wv": wvb, "msk": mask}
        )
    return in_maps


def assemble_output(results):
    out = np.empty((B, S, D), dtype=np.float32)
    ov = out.reshape(B, S // QB, QB, D)
    for c in range(NCORES):
        b, t = c // 2, c % 2
        ov[b, t::2] = results[c]["out"]
    return out


def kernel(x, Wq, Wk, Wv):
    x = np.asarray(x)
    nc = _get_program()
    in_maps = make_in_maps(x, np.asarray(Wq), np.asarray(Wk), np.asarray(Wv))
    res = run_bass_kernel_spmd(nc, in_maps, list(range(NCORES))).results
    return assemble_output(res)
